# revision 37
# baseline (speedup 1.0000x reference)
"""Trainium2 Bass kernels for BinConv2d:
   y = relu(conv2d(sign(batchnorm_train(x)), W, pad=1) + b)

Sharding: data-parallel over batch, 4 images per core on 8 cores.

Two collective-free SPMD launches (a same-launch AllReduce would pay a
30-110us rendezvous barrier + collective latency, dominated by inter-core
launch skew):
  k1: per-core BN partial sums: each core reduces its 4 images to
      per-(channel, image-pair) [128, 2] (sum x, sum x^2).
  host: folds the 8x[128,2] partials (2 KB) into per-channel thresholds
      a = gamma, d = beta*sigma - gamma*mean  (sign(xbn) == sign(a*x + d)).
  k2: streams x chunks -> binarize (exact +-1 in fp16, zero-padded
      [64ch, 114*114] per image + row-shifted duplicate on partitions
      64..127) -> 3x3 conv as 9 matmul taps: 3 K=128 kh-paired taps and 3
      K=64 taps, two 4-row output chunks concurrent on the two column
      halves of the PE array (tile_position) -> relu+bias -> store.

PE HAM warm-up: tile-position matmuls don't register as PE activity for
the clock gate, so full-array dummy matmuls run as an initial burst and
one per conv slot to hold the 2.4 GHz clock.
"""

import sys
from contextlib import ExitStack

import numpy as np

try:
    import concourse.bass as bass  # noqa: F401
except ImportError:  # pragma: no cover
    sys.path.insert(0, "/opt/trn_rl_repo")
    import concourse.bass as bass  # noqa: F401

import concourse.bacc as bacc
import concourse.tile as tile
from concourse import mybir
from concourse.bass_utils import run_bass_kernel_spmd
from concourse.masks import make_identity

F32 = mybir.dt.float32
WDT = mybir.dt.float16  # dtype for conv weights and binarized activations

N_CORES = 8
N_IMG = 4  # images per core (batch 32 / 8 cores)
NHALF = N_IMG // 2
C = 64
H = 112
W = 112
HP = H + 2  # 114
WP = W + 2  # 114
IMG = HP * WP  # 12996
EPS = 1e-4

PIX = H * W  # pixels per image plane
Q_ROWS = 28  # rows per streamed x chunk
NQ = H // Q_ROWS  # 4
QW = Q_ROWS * W  # 3136
ROWS_PER_CHUNK = 4  # output rows per matmul chunk (N = 4*112 = 448)
NMM = ROWS_PER_CHUNK * W  # 448
N_SLOTS = H // (2 * ROWS_PER_CHUNK)  # 14

N_WARM = 14  # initial full-array PE warm-up burst in k2


def build_stats_program(n_cores=N_CORES, n_img=N_IMG):
    """k1: per-core BN partial sums -> s_out [128, 2] = (sum x, sum x^2),
    partition p = 64*(n//2) + c over this core's images."""
    nc = bacc.Bacc(
        "TRN2", target_bir_lowering=False, debug=False, num_devices=n_cores
    )
    x = nc.dram_tensor("x", [n_img, C, H, W], F32, kind="ExternalInput")
    s_out = nc.dram_tensor("s_out", [128, 2], F32, kind="ExternalOutput")

    with tile.TileContext(nc) as tc, ExitStack() as ctx:
        n_chunks = NHALF * NQ
        xchp = ctx.enter_context(tc.tile_pool(name="xch", bufs=n_chunks))
        statp = ctx.enter_context(tc.tile_pool(name="stat", bufs=1))
        sums = statp.tile([128, n_chunks], F32)
        sqs = statp.tile([128, n_chunks], F32)
        sqscr = statp.tile([128, QW], F32)
        # all load triggers first so ACT Squares never block the DMA queues
        xchs = []
        for n2 in range(NHALF):
            for q in range(NQ):
                xch = xchp.tile([128, QW], F32, tag="xch")
                xchs.append(xch)
                for half in range(2):
                    n = half * NHALF + n2
                    dst = xch[half * C : half * C + C, :].rearrange(
                        "c (h w) -> c h w", w=W
                    )
                    eng = nc.sync if half == 0 else nc.scalar
                    eng.dma_start(
                        out=dst,
                        in_=x.ap()[n, :, q * Q_ROWS : (q + 1) * Q_ROWS, :],
                    )
        for idx, xch in enumerate(xchs):
            nc.vector.tensor_reduce(
                out=sums[:, idx : idx + 1],
                in_=xch,
                axis=mybir.AxisListType.X,
                op=mybir.AluOpType.add,
            )
            nc.scalar.activation(
                out=sqscr,
                in_=xch,
                func=mybir.ActivationFunctionType.Square,
                accum_out=sqs[:, idx : idx + 1],
            )
        res = statp.tile([128, 2], F32)
        nc.vector.tensor_reduce(
            out=res[:, 0:1], in_=sums,
            axis=mybir.AxisListType.X, op=mybir.AluOpType.add,
        )
        nc.vector.tensor_reduce(
            out=res[:, 1:2], in_=sqs,
            axis=mybir.AxisListType.X, op=mybir.AluOpType.add,
        )
        nc.sync.dma_start(out=s_out.ap(), in_=res)

    nc.compile()
    return nc


def build_conv_program(n_cores=N_CORES, n_img=N_IMG):
    """k2: binarize (thresholds given) + conv + relu, streaming x."""
    nc = bacc.Bacc(
        "TRN2", target_bir_lowering=False, debug=False, num_devices=n_cores
    )
    x = nc.dram_tensor("x", [n_img, C, H, W], F32, kind="ExternalInput")
    Wt = nc.dram_tensor("W", [C, C, 3, 3], F32, kind="ExternalInput")
    bt = nc.dram_tensor("b", [C], F32, kind="ExternalInput")
    av = nc.dram_tensor("avec", [C], F32, kind="ExternalInput")
    dv = nc.dram_tensor("dvec", [C], F32, kind="ExternalInput")
    y = nc.dram_tensor("y", [n_img, C, H, W], F32, kind="ExternalOutput")

    out_engines = (nc.sync, nc.scalar)

    with tile.TileContext(nc) as tc, ExitStack() as ctx:
        const = ctx.enter_context(tc.tile_pool(name="const", bufs=1))
        xchp = ctx.enter_context(tc.tile_pool(name="xch", bufs=4))
        tmpp = ctx.enter_context(tc.tile_pool(name="tmpb", bufs=2))
        xbp = ctx.enter_context(tc.tile_pool(name="xb", bufs=4))
        psump = ctx.enter_context(tc.tile_pool(name="ps", bufs=3, space="PSUM"))
        psdum = ctx.enter_context(tc.tile_pool(name="psd", bufs=3, space="PSUM"))
        pstr = ctx.enter_context(tc.tile_pool(name="pst", bufs=2, space="PSUM"))
        outp = ctx.enter_context(tc.tile_pool(name="out", bufs=4))

        # ---- constants (all const DMAs on gpsimd: keep sync/scalar for
        # the streaming loads/stores) ----
        wdum = const.tile([128, C], F32)
        nc.gpsimd.memset(wdum, 1.0)
        wcst = const.tile([128, NMM], F32)
        nc.gpsimd.memset(wcst, 1.0)
        identity64 = const.tile([C, C], F32)
        make_identity(nc, identity64)

        dum_i = 0

        def dummy_mm(rhs=None):
            nonlocal dum_i
            psD = psdum.tile([C, NMM], F32, tag="psd")
            nc.tensor.matmul(
                psD,
                wdum,
                wcst if rhs is None else rhs,
                start=True,
                stop=True,
                skip_group_check=True,
            )
            dum_i += 1

        # initial full-array warm-up burst (no data deps -> runs at t~0)
        for _ in range(N_WARM):
            dummy_mm()

        # W loads contiguously as [o, (c kh kw)]; per-tap 64x64 PE
        # transposes produce lhsT[c, o], cast to fp16.
        wsb = const.tile([C, C, 9], F32)
        nc.gpsimd.dma_start(
            out=wsb, in_=Wt.ap().rearrange("o c kh kw -> o c (kh kw)")
        )
        w2 = const.tile([128, 9, C], WDT)
        for t in range(9):
            psT = pstr.tile([C, C], F32, tag="pst")
            nc.tensor.transpose(psT, wsb[:, :, t], identity64)
            nc.scalar.activation(
                out=w2[0:C, t, :], in_=psT,
                func=mybir.ActivationFunctionType.Copy,
            )
            if t >= 3:
                nc.scalar.activation(
                    out=w2[C:128, t - 3, :], in_=psT,
                    func=mybir.ActivationFunctionType.Copy,
                )
        b2 = const.tile([128, 1], F32)
        bsrc = bt.ap().rearrange("(c u) -> c u", u=1)
        nc.gpsimd.dma_start(out=b2[0:C, :], in_=bsrc)
        nc.gpsimd.dma_start(out=b2[C:128, :], in_=bsrc)
        a2 = const.tile([128, 1], F32)
        asrc = av.ap().rearrange("(c u) -> c u", u=1)
        nc.gpsimd.dma_start(out=a2[0:C, :], in_=asrc)
        nc.gpsimd.dma_start(out=a2[C:128, :], in_=asrc)
        d2 = const.tile([128, 1], F32)
        dsrc = dv.ap().rearrange("(c u) -> c u", u=1)
        nc.gpsimd.dma_start(out=d2[0:C, :], in_=dsrc)
        nc.gpsimd.dma_start(out=d2[C:128, :], in_=dsrc)

        out_dma_i = 0

        def conv_slot(n, xbv, s):
            nonlocal out_dma_i
            h0 = s * 2 * ROWS_PER_CHUNK
            h1 = h0 + ROWS_PER_CHUNK
            dummy_mm()  # hold the HAM clock warm (tile_position mms don't)
            P = psump.tile([128, NMM], F32, tag="psum")
            mms = []
            for kw in range(3):
                for cg, hb in ((0, h0), (64, h1)):
                    mms.append((cg, hb, kw, True))
            for kw in range(3):
                for cg, hb in ((0, h0), (64, h1)):
                    mms.append((cg, hb, kw, False))
            cg_seen = set()
            cg_last = {cg: max(i for i, m in enumerate(mms) if m[0] == cg)
                       for cg in (0, 64)}
            for i, (cg, hb, kw, is_pair) in enumerate(mms):
                if is_pair:
                    lhsT = w2[:, kw, :]
                    rhs = xbv[:, hb : hb + ROWS_PER_CHUNK, kw : kw + W]
                else:
                    lhsT = w2[0:C, 6 + kw, :]
                    rhs = xbv[0:C, hb + 2 : hb + 2 + ROWS_PER_CHUNK, kw : kw + W]
                nc.tensor.matmul(
                    P[cg : cg + C, :],
                    lhsT,
                    rhs,
                    start=(cg not in cg_seen),
                    stop=(i == cg_last[cg]),
                    tile_position=(0, cg),
                    skip_group_check=True,
                )
                cg_seen.add(cg)
            # epilogue relu(P + b): alternate ACT/DVE to balance engines
            osb = outp.tile([128, NMM], F32, tag="osb")
            if s % 2 == 0:
                nc.scalar.activation(
                    out=osb,
                    in_=P,
                    func=mybir.ActivationFunctionType.Relu,
                    bias=b2,
                )
            else:
                nc.vector.tensor_scalar(
                    out=osb,
                    in0=P,
                    scalar1=b2,
                    scalar2=0.0,
                    op0=mybir.AluOpType.add,
                    op1=mybir.AluOpType.max,
                )
            ov = osb.rearrange("p (h w) -> p h w", w=W)
            e0 = out_engines[out_dma_i % 2]
            e1 = out_engines[(out_dma_i + 1) % 2]
            out_dma_i += 2
            e0.dma_start(
                out=y.ap()[n, :, h0 : h0 + ROWS_PER_CHUNK, :],
                in_=ov[0:C, :, :],
            )
            e1.dma_start(
                out=y.ap()[n, :, h1 : h1 + ROWS_PER_CHUNK, :],
                in_=ov[C:128, :, :],
            )

        # ---- stream: per image-pair (n2, n2+2), per 28-row chunk ----
        # conv slots of image X unlock after chunk q: s <= (28q+19)//8
        slot_hi = [(Q_ROWS * (q + 1) - 9) // 8 for q in range(NQ)]
        slot_hi[-1] = N_SLOTS - 1
        for n2 in range(NHALF):
            imgs = (n2, NHALF + n2)
            xbts, xbvs = [], []
            for n in imgs:
                xbt = xbp.tile([128, IMG], WDT, tag="xb")
                xbv = xbt.rearrange("p (hp wp) -> p hp wp", wp=WP)
                xbts.append(xbt)
                xbvs.append(xbv)
                nc.gpsimd.memset(xbv[0:C, 0:1, :], 0.0)
                nc.gpsimd.memset(xbv[0:C, HP - 1 : HP, :], 0.0)
                nc.gpsimd.memset(xbv[0:C, 1 : HP - 1, 0:1], 0.0)
                nc.gpsimd.memset(xbv[0:C, 1 : HP - 1, WP - 1 : WP], 0.0)
            # all 4 chunk-loads up front so ACT work never blocks triggers
            xchs = []
            for q in range(NQ):
                xch = xchp.tile([128, QW], F32, tag="xch")
                xchs.append(xch)
                for half in range(2):
                    eng = nc.sync if half == 0 else nc.scalar
                    eng.dma_start(
                        out=xch[half * C : half * C + C, :].rearrange(
                            "c (h w) -> c h w", w=W
                        ),
                        in_=x.ap()[
                            imgs[half], :, q * Q_ROWS : (q + 1) * Q_ROWS, :
                        ],
                    )
            if n2 == 0:
                # bridge the PE warm-up until conv slots start: dummies
                # gated on arriving chunks run back-to-back with the burst
                for k in range(8):
                    dummy_mm(rhs=xchs[0][:, (k % 7) * NMM : (k % 7) * NMM + NMM])
                for k in range(4):
                    dummy_mm(rhs=xchs[1][:, k * NMM : k * NMM + NMM])
            slot_done = [0, 0]
            for q in range(NQ):
                xch = xchs[q]
                h0c = q * Q_ROWS
                h1c = (q + 1) * Q_ROWS
                # binarize both images in one ACT pass: tmpb = Sign(a*x+d)
                tmpb = tmpp.tile([128, QW], WDT, tag="tmpb")
                nc.scalar.activation(
                    out=tmpb,
                    in_=xch,
                    func=mybir.ActivationFunctionType.Sign,
                    scale=a2,
                    bias=d2,
                )
                for half in range(2):
                    # distribute into padded copy A (DVE strided write)
                    nc.vector.tensor_copy(
                        out=xbvs[half][0:C, 1 + h0c : 1 + h1c, 1 : WP - 1],
                        in_=tmpb[half * C : half * C + C, :].rearrange(
                            "c (h w) -> c h w", w=W
                        ),
                    )
                    # copy B rows = A rows +1: chunk q provides A rows
                    # 1+h0c..h1c, so B rows h0c-1(+)..h1c-1; last chunk
                    # extends through row 112 (A row 113 is zero border)
                    lo = 0 if q == 0 else (h0c - 1) * WP
                    hi_ = (h1c - 1) * WP if h1c < H else IMG - WP
                    nc.vector.tensor_copy(
                        out=xbts[half][C:128, lo:hi_],
                        in_=xbts[half][0:C, lo + WP : hi_ + WP],
                    )
                for half in range(2):
                    for s in range(slot_done[half], slot_hi[q] + 1):
                        conv_slot(imgs[half], xbvs[half], s)
                    slot_done[half] = slot_hi[q] + 1

    nc.compile()
    return nc


_CACHE = {}


def _get_programs(n_cores=N_CORES, n_img=N_IMG):
    key = (n_cores, n_img)
    if key not in _CACHE:
        _CACHE[key] = (
            build_stats_program(n_cores, n_img),
            build_conv_program(n_cores, n_img),
        )
    return _CACHE[key]


def kernel(x, gamma, beta, W, b, _trace=False):
    x = np.ascontiguousarray(x, dtype=np.float32)
    gamma = np.ascontiguousarray(gamma, np.float32)
    beta = np.ascontiguousarray(beta, np.float32)
    W = np.ascontiguousarray(W, np.float32)
    b = np.ascontiguousarray(b, np.float32)
    assert x.shape[0] == N_CORES * N_IMG, x.shape
    nc1, nc2 = _get_programs(N_CORES, N_IMG)

    shards = [x[c * N_IMG : (c + 1) * N_IMG] for c in range(N_CORES)]
    res1 = run_bass_kernel_spmd(
        nc1,
        [{"x": s} for s in shards],
        core_ids=list(range(N_CORES)),
        trace=_trace,
    )
    # fold partials -> per-channel thresholds (float64 host math, 2 KB)
    parts = np.stack([res1.results[c]["s_out"] for c in range(N_CORES)])
    tot = parts.astype(np.float64).sum(axis=0)  # [128, 2]
    tot64 = tot[:C] + tot[C:]  # fold image-pair halves -> [64, 2]
    count = float(N_CORES * N_IMG * PIX)
    mean = tot64[:, 0] / count
    var = tot64[:, 1] / count - mean * mean
    sigma = np.sqrt(var + EPS)
    avec = gamma.astype(np.float64)
    dvec = beta.astype(np.float64) * sigma - avec * mean
    avec = avec.astype(np.float32)
    dvec = dvec.astype(np.float32)

    res2 = run_bass_kernel_spmd(
        nc2,
        [
            {"x": s, "W": W, "b": b, "avec": avec, "dvec": dvec}
            for s in shards
        ],
        core_ids=list(range(N_CORES)),
        trace=_trace,
    )
    out = np.concatenate([res2.results[c]["y"] for c in range(N_CORES)], axis=0)
    if _trace:
        kernel._last_result = (res1, res2)
    return out
